# revision 35
# baseline (speedup 1.0000x reference)
"""AR(64) sampling kernel for Trainium2 (8 NeuronCores, batch-sharded).

Problem: x_t = sum_k c_k x_{t-64+k} + sigma * eps_t over 4096 steps for
16384 independent batch rows (64 lags).

Approach: the recurrence is linear, so a block of 64 consecutive outputs
is an exact linear function of (the previous 64 outputs, the block's 64
noise values):

    y_block = AS @ state + (sigma*AE) @ eps        (per batch column)

AS/AE are built on the host from the coefficients by running the
recurrence with unit initial conditions / unit impulses (exact linear
algebra, a few thousand host flops). On device, each core processes
2048 batch rows (time-major layout) as a chain of 64 blocks x 4
batch-chunks of 512: per (block, chunk) one K=128 float32r PE matmul
(lhsT = [AS; sigma*AE], rhs = [state; noise] assembled in one SBUF
tile), then a PSUM->SBUF copy of the result (which is simultaneously
the next block's state rows and the DMA-out staging), with streaming
DMA of noise in / outputs out.

float32r runs the PE single-pass (4x the fp32 matmul rate; fp32 needs 2
half-speed passes) at a 12-bit-mantissa input rounding (rel ~1.2e-4);
measured end-to-end error is ~7e-4 of the output absmax vs the fp32
reference, while fp32's ~6e-7 costs ~15% more wall time - this kernel
is DMA-bound either way (~225 GB/s aggregate measured per core).

DMA layout: noise / outputs live in DRAM lane-major ([64 lane, n_blocks,
Bc]) so every transfer is 8KB contiguous per SBUF partition. Loads ride
the SP HW-DGE ring, stores the ACT ring.
"""

import sys

import numpy as np

_TRN_REPO = "/opt/trn_rl_repo"
if _TRN_REPO not in sys.path:
    sys.path.insert(0, _TRN_REPO)

_TB = 64  # time-block size == number of AR lags
_NCORES = 8


def _build_weights(coefficients: np.ndarray, sigma: float) -> np.ndarray:
    """Exact [2n, n] block-transition weights from AR coefficients.

    Returns lhsT with lhsT.T @ [state; eps] = y_block, where state is the
    previous 64 outputs (oldest first) and eps the block's raw noise.
    """
    c = np.asarray(coefficients, dtype=np.float64)
    n = c.shape[0]
    assert n == _TB

    # AS[i, r] = d y_i / d state_r : simulate with window = unit vectors.
    win = np.eye(n, dtype=np.float64)  # rows: unit-state cases
    AS = np.empty((_TB, n), dtype=np.float64)
    for i in range(_TB):
        x = win @ c
        AS[i] = x
        win = np.concatenate([win[:, 1:], x[:, None]], axis=1)

    # AE[i, j] = d y_i / d eps_j : simulate unit impulses, zero init.
    win = np.zeros((_TB, n), dtype=np.float64)
    AE = np.empty((_TB, _TB), dtype=np.float64)
    for i in range(_TB):
        x = win @ c
        x[i] += 1.0
        AE[i] = x
        win = np.concatenate([win[:, 1:], x[:, None]], axis=1)

    W = np.concatenate([AS.T, float(sigma) * AE.T], axis=0)  # [2n, TB]
    return np.ascontiguousarray(W.astype(np.float32))


def blocked_numpy(initial_values, coefficients, log_noise_std, noise):
    """Host-side blocked simulation (same math the device runs); for testing."""
    sigma = float(np.exp(np.float64(np.asarray(log_noise_std))))
    W = _build_weights(coefficients, sigma)
    B, T = noise.shape
    y = np.empty((B, T), np.float32)
    state = np.asarray(initial_values, np.float32)
    for b in range(T // _TB):
        rhs = np.concatenate([state.T, noise[:, b * _TB:(b + 1) * _TB].T], axis=0)
        out = (W.T @ rhs.astype(np.float32)).astype(np.float32)  # [TB, B]
        y[:, b * _TB:(b + 1) * _TB] = out.T
        state = out.T
    return y


def _round_f32r(x: np.ndarray) -> np.ndarray:
    """Round fp32 values to the fp32r matmul datapath precision (mantissa
    rounded at bit 12), round-to-nearest-even - bit-exact vs walrus's
    fp32_to_fp32r."""
    u = np.ascontiguousarray(x, np.float32).view(np.uint32)
    low = u & np.uint32(0xFFF)
    base = u >> np.uint32(12)
    add = (low > 0x800) | ((low == 0x800) & ((base & 1) == 1))
    r = ((base + add.astype(np.uint32)) << np.uint32(12)).astype(np.uint32)
    return r.view(np.float32)


def _build_nc(T: int, Bc: int, chunk: int, bufs: int = 10):
    """Build the per-core Bass/Tile program.

    DRAM tensors:
      noise_h [64, nb, Bc] fp16   - lane-major noise
      init_t  [64, Bc]     fp32r  - initial window, lane-major (oldest first)
      w       [128, 64]    fp32r  - block-transition weights (lhsT)
      y_d     [64, nb, Bc] fp32r  - lane-major outputs
    """
    from concourse import bacc
    import concourse.mybir as mybir
    from concourse.tile import TileContext

    assert T % _TB == 0 and Bc % chunk == 0
    nb = T // _TB
    nchunks = Bc // chunk

    nc = bacc.Bacc("TRN2", target_bir_lowering=False, debug=False)
    f32 = mybir.dt.float32
    f32r = mybir.dt.float32r
    f16 = mybir.dt.float16
    noise_h = nc.dram_tensor("noise_h", [_TB, nb, Bc], f16, kind="ExternalInput")
    init_t = nc.dram_tensor("init_t", [_TB, Bc], f32r, kind="ExternalInput")
    w = nc.dram_tensor("w", [2 * _TB, _TB], f32r, kind="ExternalInput")
    y_d = nc.dram_tensor("y_d", [_TB, nb, Bc], f32r, kind="ExternalOutput")

    with TileContext(nc) as tc:
        with tc.tile_pool(name="wpool", bufs=1) as wpool, \
             tc.tile_pool(name="rhs", bufs=bufs) as rhspool, \
             tc.tile_pool(name="stage", bufs=bufs) as stagepool, \
             tc.tile_pool(name="ps", bufs=2, space="PSUM") as pspool:
            wt = wpool.tile([2 * _TB, _TB], f32r, tag="wt", name="wt")
            nc.sync.dma_start(out=wt[:, :], in_=w[:, :])

            # One rhs tile per block [128, Bc]: rows 0:64 = state (previous
            # block's outputs, also the store staging), rows 64:128 = this
            # block's noise. The noise arrives fp16 (halves the load-stream
            # bytes) into a staging tile on the SP HW-DGE ring; DVE/ACT
            # cast-copy it to fp32r three blocks ahead of use, in [64,512]
            # pieces interleaved one-per-chain-copy so a conversion never
            # delays the chain-critical PSUM->SBUF copies by more than
            # ~0.5us in the engine FIFOs.
            def alloc_rhs(b):
                t = rhspool.tile([2 * _TB, Bc], f32r, tag="rhs", name="rhs")
                st = None
                if b < nb:
                    st = stagepool.tile([2 * _TB, Bc], f16, tag="stage",
                                        name="stage")
                    nc.sync.dma_start(out=st[_TB:, :], in_=noise_h[:, b, :])
                return t, st

            tiles, stages = {}, {}
            for i in range(3):
                tiles[i], stages[i] = alloc_rhs(i)
            nc.sync.dma_start(out=tiles[0][0:_TB, :], in_=init_t[:, :])

            def emit_conv_piece(b, c):
                cs = slice(c * chunk, (c + 1) * chunk)
                src = stages[b][_TB:, cs]
                dst = tiles[b][_TB:, cs]
                # 3:1 toward DVE - the ACT stream is the tighter engine.
                if c != 3:
                    nc.vector.tensor_copy(out=dst, in_=src)
                else:
                    nc.scalar.copy(out=dst, in_=src)

            for bb in range(3):
                for c in range(nchunks):
                    emit_conv_piece(bb, c)

            for b in range(nb):
                cur = tiles[b]
                if b + 3 <= nb:
                    tiles[b + 3], stages[b + 3] = alloc_rhs(b + 3)
                nxt = tiles[b + 1]
                pss = []
                for c in range(nchunks):
                    cs = slice(c * chunk, (c + 1) * chunk)
                    ps = pspool.tile([_TB, chunk], f32, tag=f"ps{c}",
                                     name=f"ps{c}")
                    # Single K=128 float32r matmul (PE single-pass).
                    nc.tensor.matmul(
                        out=ps[:, :], lhsT=wt[:, :], rhs=cur[:, cs],
                        start=True, stop=True,
                    )
                    pss.append(ps)
                for c in range(nchunks):
                    cs = slice(c * chunk, (c + 1) * chunk)
                    # Chain-critical PSUM->SBUF copy, split across DVE/ACT.
                    if c % 2 == 0:
                        nc.vector.tensor_copy(out=nxt[0:_TB, cs],
                                              in_=pss[c][:, :])
                    else:
                        nc.scalar.copy(out=nxt[0:_TB, cs], in_=pss[c][:, :])
                    # One conversion piece for block b+3 behind it.
                    if b + 3 < nb:
                        emit_conv_piece(b + 3, c)
                # Store block b's outputs (staged as block b+1's state rows).
                # Issue via the GpSimd SW-DGE ring: a third queue row, so
                # store packets interleave with the SP-ring loads at the
                # engines instead of serializing behind them, and the ~1us
                # descriptor-gen cost lands on the otherwise-idle POOL
                # sequencer rather than ACT/DVE/SP.
                nc.gpsimd.dma_start(out=y_d[:, b, :], in_=nxt[0:_TB, :])
                del tiles[b]
                stages.pop(b, None)

    nc.compile()
    return nc


def _shard_inputs(initial_values, coefficients, log_noise_std, noise):
    B, T = noise.shape
    Bc = B // _NCORES
    nb = T // _TB
    sigma = float(np.exp(np.float64(np.asarray(log_noise_std))))
    W = _round_f32r(_build_weights(coefficients, sigma))
    noise_tf = np.asarray(noise, np.float32).T.astype(np.float16)  # [T, B]
    init_tf = _round_f32r(np.asarray(initial_values, np.float32).T)
    # lane-major: [T, B] -> [nb, 64, B] -> [64, nb, B]
    noise_lane = np.ascontiguousarray(
        noise_tf.reshape(nb, _TB, B).transpose(1, 0, 2))
    in_maps = []
    for i in range(_NCORES):
        cols = slice(i * Bc, (i + 1) * Bc)
        in_maps.append({
            "noise_h": np.ascontiguousarray(noise_lane[:, :, cols]),
            "init_t": np.ascontiguousarray(init_tf[:, cols]),
            "w": W,
        })
    return in_maps


def _run(initial_values, coefficients, log_noise_std, noise, trace=False):
    from concourse.bass_utils import run_bass_kernel_spmd

    B, T = noise.shape
    Bc = B // _NCORES
    chunk = 512 if Bc % 512 == 0 else Bc
    nc = _build_nc(T, Bc, chunk)
    in_maps = _shard_inputs(initial_values, coefficients, log_noise_std, noise)
    res = run_bass_kernel_spmd(
        nc, in_maps, core_ids=list(range(_NCORES)), trace=trace
    )
    # y_d [64, nb, Bc] lane-major -> [T, Bc]
    y_cores = [
        r["y_d"].transpose(1, 0, 2).reshape(T, Bc) for r in res.results
    ]
    y_t = np.concatenate(y_cores, axis=1)  # [T, B]
    out = np.ascontiguousarray(y_t.T)
    return out, res


def kernel(initial_values, coefficients, log_noise_std, noise, steps):
    steps = int(np.asarray(steps))
    noise = np.asarray(noise)
    assert noise.shape[1] == steps, (noise.shape, steps)
    out, _ = _run(initial_values, coefficients, log_noise_std, noise)
    return out


# revision 36
# speedup vs baseline: 1.0349x; 1.0349x over previous
"""AR(64) sampling kernel for Trainium2 (8 NeuronCores, batch-sharded).

Problem: x_t = sum_k c_k x_{t-64+k} + sigma * eps_t over 4096 steps for
16384 independent batch rows (64 lags).

Approach: the recurrence is linear, so a block of 64 consecutive outputs
is an exact linear function of (the previous 64 outputs, the block's 64
noise values):

    y_block = AS @ state + (sigma*AE) @ eps        (per batch column)

AS/AE are built on the host from the coefficients by running the
recurrence with unit initial conditions / unit impulses (exact linear
algebra, a few thousand host flops). On device, each core processes
2048 batch rows (time-major layout) as a chain of 64 blocks x 4
batch-chunks of 512: per (block, chunk) one K=128 float32r PE matmul
(lhsT = [AS; sigma*AE], rhs = [state; noise] assembled in one SBUF
tile), then a PSUM->SBUF copy of the result (which is simultaneously
the next block's state rows and the DMA-out staging), with streaming
DMA of noise in / outputs out.

float32r runs the PE single-pass (4x the fp32 matmul rate; fp32 needs 2
half-speed passes) at a 12-bit-mantissa input rounding (rel ~1.2e-4);
measured end-to-end error is ~7e-4 of the output absmax vs the fp32
reference, while fp32's ~6e-7 costs ~15% more wall time - this kernel
is DMA-bound either way (~225 GB/s aggregate measured per core).

DMA layout: noise / outputs live in DRAM lane-major ([64 lane, n_blocks,
Bc]) so every transfer is 8KB contiguous per SBUF partition. Loads ride
the SP HW-DGE ring, stores the ACT ring.
"""

import sys

import numpy as np

_TRN_REPO = "/opt/trn_rl_repo"
if _TRN_REPO not in sys.path:
    sys.path.insert(0, _TRN_REPO)

_TB = 64  # time-block size == number of AR lags
_NCORES = 8


def _build_weights(coefficients: np.ndarray, sigma: float) -> np.ndarray:
    """Exact [2n, n] block-transition weights from AR coefficients.

    Returns lhsT with lhsT.T @ [state; eps] = y_block, where state is the
    previous 64 outputs (oldest first) and eps the block's raw noise.
    """
    c = np.asarray(coefficients, dtype=np.float64)
    n = c.shape[0]
    assert n == _TB

    # AS[i, r] = d y_i / d state_r : simulate with window = unit vectors.
    win = np.eye(n, dtype=np.float64)  # rows: unit-state cases
    AS = np.empty((_TB, n), dtype=np.float64)
    for i in range(_TB):
        x = win @ c
        AS[i] = x
        win = np.concatenate([win[:, 1:], x[:, None]], axis=1)

    # AE[i, j] = d y_i / d eps_j : simulate unit impulses, zero init.
    win = np.zeros((_TB, n), dtype=np.float64)
    AE = np.empty((_TB, _TB), dtype=np.float64)
    for i in range(_TB):
        x = win @ c
        x[i] += 1.0
        AE[i] = x
        win = np.concatenate([win[:, 1:], x[:, None]], axis=1)

    W = np.concatenate([AS.T, float(sigma) * AE.T], axis=0)  # [2n, TB]
    return np.ascontiguousarray(W.astype(np.float32))


def blocked_numpy(initial_values, coefficients, log_noise_std, noise):
    """Host-side blocked simulation (same math the device runs); for testing."""
    sigma = float(np.exp(np.float64(np.asarray(log_noise_std))))
    W = _build_weights(coefficients, sigma)
    B, T = noise.shape
    y = np.empty((B, T), np.float32)
    state = np.asarray(initial_values, np.float32)
    for b in range(T // _TB):
        rhs = np.concatenate([state.T, noise[:, b * _TB:(b + 1) * _TB].T], axis=0)
        out = (W.T @ rhs.astype(np.float32)).astype(np.float32)  # [TB, B]
        y[:, b * _TB:(b + 1) * _TB] = out.T
        state = out.T
    return y


def _round_f32r(x: np.ndarray) -> np.ndarray:
    """Round fp32 values to the fp32r matmul datapath precision (mantissa
    rounded at bit 12), round-to-nearest-even - bit-exact vs walrus's
    fp32_to_fp32r."""
    u = np.ascontiguousarray(x, np.float32).view(np.uint32)
    low = u & np.uint32(0xFFF)
    base = u >> np.uint32(12)
    add = (low > 0x800) | ((low == 0x800) & ((base & 1) == 1))
    r = ((base + add.astype(np.uint32)) << np.uint32(12)).astype(np.uint32)
    return r.view(np.float32)


def _build_nc(T: int, Bc: int, chunk: int, bufs: int = 8):
    """Build the per-core Bass/Tile program.

    DRAM tensors:
      noise_h [64, nb, Bc] fp16   - lane-major noise
      init_t  [64, Bc]     fp32r  - initial window, lane-major (oldest first)
      w       [128, 64]    fp32r  - block-transition weights (lhsT)
      y_d     [64, nb, Bc] fp32r  - lane-major outputs
    """
    from concourse import bacc
    import concourse.mybir as mybir
    from concourse.tile import TileContext

    assert T % _TB == 0 and Bc % chunk == 0
    nb = T // _TB
    nchunks = Bc // chunk

    nc = bacc.Bacc("TRN2", target_bir_lowering=False, debug=False)
    f32 = mybir.dt.float32
    f32r = mybir.dt.float32r
    f16 = mybir.dt.float16
    noise_h = nc.dram_tensor("noise_h", [_TB, nb, Bc], f16, kind="ExternalInput")
    init_t = nc.dram_tensor("init_t", [_TB, Bc], f32r, kind="ExternalInput")
    w = nc.dram_tensor("w", [2 * _TB, _TB], f32r, kind="ExternalInput")
    y_d = nc.dram_tensor("y_d", [_TB, nb, Bc], f32r, kind="ExternalOutput")

    with TileContext(nc) as tc:
        with tc.tile_pool(name="wpool", bufs=1) as wpool, \
             tc.tile_pool(name="rhs", bufs=bufs) as rhspool, \
             tc.tile_pool(name="stage", bufs=bufs) as stagepool, \
             tc.tile_pool(name="ps", bufs=2, space="PSUM") as pspool:
            wt = wpool.tile([2 * _TB, _TB], f32r, tag="wt", name="wt")
            nc.sync.dma_start(out=wt[:, :], in_=w[:, :])

            # One rhs tile per block [128, Bc]: rows 0:64 = state (previous
            # block's outputs, also the store staging), rows 64:128 = this
            # block's noise. The noise arrives fp16 (halves the load-stream
            # bytes) into a staging tile on the SP HW-DGE ring; DVE/ACT
            # cast-copy it to fp32r three blocks ahead of use, in [64,512]
            # pieces interleaved one-per-chain-copy so a conversion never
            # delays the chain-critical PSUM->SBUF copies by more than
            # ~0.5us in the engine FIFOs.
            def alloc_rhs(b):
                t = rhspool.tile([2 * _TB, Bc], f32r, tag="rhs", name="rhs")
                st = None
                if b < nb:
                    st = stagepool.tile([2 * _TB, Bc], f16, tag="stage",
                                        name="stage")
                    nc.sync.dma_start(out=st[_TB:, :], in_=noise_h[:, b, :])
                return t, st

            tiles, stages = {}, {}
            for i in range(3):
                tiles[i], stages[i] = alloc_rhs(i)
            nc.sync.dma_start(out=tiles[0][0:_TB, :], in_=init_t[:, :])

            def emit_conv_piece(b, c):
                cs = slice(c * chunk, (c + 1) * chunk)
                src = stages[b][_TB:, cs]
                dst = tiles[b][_TB:, cs]
                if c % 2 == 0:
                    nc.vector.tensor_copy(out=dst, in_=src)
                else:
                    nc.scalar.copy(out=dst, in_=src)

            for bb in range(3):
                for c in range(nchunks):
                    emit_conv_piece(bb, c)

            for b in range(nb):
                cur = tiles[b]
                if b + 3 <= nb:
                    tiles[b + 3], stages[b + 3] = alloc_rhs(b + 3)
                nxt = tiles[b + 1]
                pss = []
                for c in range(nchunks):
                    cs = slice(c * chunk, (c + 1) * chunk)
                    ps = pspool.tile([_TB, chunk], f32, tag=f"ps{c}",
                                     name=f"ps{c}")
                    # Single K=128 float32r matmul (PE single-pass).
                    nc.tensor.matmul(
                        out=ps[:, :], lhsT=wt[:, :], rhs=cur[:, cs],
                        start=True, stop=True,
                    )
                    pss.append(ps)
                for c in range(nchunks):
                    cs = slice(c * chunk, (c + 1) * chunk)
                    # Chain-critical PSUM->SBUF copy, split across DVE/ACT.
                    if c % 2 == 0:
                        nc.vector.tensor_copy(out=nxt[0:_TB, cs],
                                              in_=pss[c][:, :])
                    else:
                        nc.scalar.copy(out=nxt[0:_TB, cs], in_=pss[c][:, :])
                    # One conversion piece for block b+3 behind it.
                    if b + 3 < nb:
                        emit_conv_piece(b + 3, c)
                # Store block b's outputs (staged as block b+1's state rows).
                # Issue from the otherwise-idle SP sequencer to keep the
                # copy engines' instruction streams short.
                nc.sync.dma_start(out=y_d[:, b, :], in_=nxt[0:_TB, :])
                del tiles[b]
                stages.pop(b, None)

    nc.compile()
    return nc


def _shard_inputs(initial_values, coefficients, log_noise_std, noise):
    B, T = noise.shape
    Bc = B // _NCORES
    nb = T // _TB
    sigma = float(np.exp(np.float64(np.asarray(log_noise_std))))
    W = _round_f32r(_build_weights(coefficients, sigma))
    noise_tf = np.asarray(noise, np.float32).T.astype(np.float16)  # [T, B]
    init_tf = _round_f32r(np.asarray(initial_values, np.float32).T)
    # lane-major: [T, B] -> [nb, 64, B] -> [64, nb, B]
    noise_lane = np.ascontiguousarray(
        noise_tf.reshape(nb, _TB, B).transpose(1, 0, 2))
    in_maps = []
    for i in range(_NCORES):
        cols = slice(i * Bc, (i + 1) * Bc)
        in_maps.append({
            "noise_h": np.ascontiguousarray(noise_lane[:, :, cols]),
            "init_t": np.ascontiguousarray(init_tf[:, cols]),
            "w": W,
        })
    return in_maps


def _run(initial_values, coefficients, log_noise_std, noise, trace=False):
    from concourse.bass_utils import run_bass_kernel_spmd

    B, T = noise.shape
    Bc = B // _NCORES
    chunk = 512 if Bc % 512 == 0 else Bc
    nc = _build_nc(T, Bc, chunk)
    in_maps = _shard_inputs(initial_values, coefficients, log_noise_std, noise)
    res = run_bass_kernel_spmd(
        nc, in_maps, core_ids=list(range(_NCORES)), trace=trace
    )
    # y_d [64, nb, Bc] lane-major -> [T, Bc]
    y_cores = [
        r["y_d"].transpose(1, 0, 2).reshape(T, Bc) for r in res.results
    ]
    y_t = np.concatenate(y_cores, axis=1)  # [T, B]
    out = np.ascontiguousarray(y_t.T)
    return out, res


def kernel(initial_values, coefficients, log_noise_std, noise, steps):
    steps = int(np.asarray(steps))
    noise = np.asarray(noise)
    assert noise.shape[1] == steps, (noise.shape, steps)
    out, _ = _run(initial_values, coefficients, log_noise_std, noise)
    return out


# revision 37
# speedup vs baseline: 1.0356x; 1.0007x over previous
"""AR(64) sampling kernel for Trainium2 (8 NeuronCores, batch-sharded).

Problem: x_t = sum_k c_k x_{t-64+k} + sigma * eps_t over 4096 steps for
16384 independent batch rows (64 lags).

Approach: the recurrence is linear, so a block of 64 consecutive outputs
is an exact linear function of (the previous 64 outputs, the block's 64
noise values):

    y_block = AS @ state + (sigma*AE) @ eps        (per batch column)

AS/AE are built on the host from the coefficients by running the
recurrence with unit initial conditions / unit impulses (exact linear
algebra, a few thousand host flops). On device, each core processes
2048 batch rows (time-major layout) as a chain of 64 blocks x 4
batch-chunks of 512: per (block, chunk) one K=128 float32r PE matmul
(lhsT = [AS; sigma*AE], rhs = [state; noise] assembled in one SBUF
tile), then a PSUM->SBUF copy of the result (which is simultaneously
the next block's state rows and the DMA-out staging), with streaming
DMA of noise in / outputs out.

float32r runs the PE single-pass (4x the fp32 matmul rate; fp32 needs 2
half-speed passes) at a 12-bit-mantissa input rounding (rel ~1.2e-4);
measured end-to-end error is ~7e-4 of the output absmax vs the fp32
reference, while fp32's ~6e-7 costs ~15% more wall time - this kernel
is DMA-bound either way (~225 GB/s aggregate measured per core).

DMA layout: noise / outputs live in DRAM lane-major ([64 lane, n_blocks,
Bc]) so every transfer is 8KB contiguous per SBUF partition. Loads ride
the SP HW-DGE ring, stores the ACT ring.
"""

import sys

import numpy as np

_TRN_REPO = "/opt/trn_rl_repo"
if _TRN_REPO not in sys.path:
    sys.path.insert(0, _TRN_REPO)

_TB = 64  # time-block size == number of AR lags
_NCORES = 8


def _build_weights(coefficients: np.ndarray, sigma: float) -> np.ndarray:
    """Exact [2n, n] block-transition weights from AR coefficients.

    Returns lhsT with lhsT.T @ [state; eps] = y_block, where state is the
    previous 64 outputs (oldest first) and eps the block's raw noise.
    """
    c = np.asarray(coefficients, dtype=np.float64)
    n = c.shape[0]
    assert n == _TB

    # AS[i, r] = d y_i / d state_r : simulate with window = unit vectors.
    win = np.eye(n, dtype=np.float64)  # rows: unit-state cases
    AS = np.empty((_TB, n), dtype=np.float64)
    for i in range(_TB):
        x = win @ c
        AS[i] = x
        win = np.concatenate([win[:, 1:], x[:, None]], axis=1)

    # AE[i, j] = d y_i / d eps_j : simulate unit impulses, zero init.
    win = np.zeros((_TB, n), dtype=np.float64)
    AE = np.empty((_TB, _TB), dtype=np.float64)
    for i in range(_TB):
        x = win @ c
        x[i] += 1.0
        AE[i] = x
        win = np.concatenate([win[:, 1:], x[:, None]], axis=1)

    W = np.concatenate([AS.T, float(sigma) * AE.T], axis=0)  # [2n, TB]
    return np.ascontiguousarray(W.astype(np.float32))


def blocked_numpy(initial_values, coefficients, log_noise_std, noise):
    """Host-side blocked simulation (same math the device runs); for testing."""
    sigma = float(np.exp(np.float64(np.asarray(log_noise_std))))
    W = _build_weights(coefficients, sigma)
    B, T = noise.shape
    y = np.empty((B, T), np.float32)
    state = np.asarray(initial_values, np.float32)
    for b in range(T // _TB):
        rhs = np.concatenate([state.T, noise[:, b * _TB:(b + 1) * _TB].T], axis=0)
        out = (W.T @ rhs.astype(np.float32)).astype(np.float32)  # [TB, B]
        y[:, b * _TB:(b + 1) * _TB] = out.T
        state = out.T
    return y


def _round_f32r(x: np.ndarray) -> np.ndarray:
    """Round fp32 values to the fp32r matmul datapath precision (mantissa
    rounded at bit 12), round-to-nearest-even - bit-exact vs walrus's
    fp32_to_fp32r."""
    u = np.ascontiguousarray(x, np.float32).view(np.uint32)
    low = u & np.uint32(0xFFF)
    base = u >> np.uint32(12)
    add = (low > 0x800) | ((low == 0x800) & ((base & 1) == 1))
    r = ((base + add.astype(np.uint32)) << np.uint32(12)).astype(np.uint32)
    return r.view(np.float32)


def _build_nc(T: int, Bc: int, chunk: int, bufs: int = 8):
    """Build the per-core Bass/Tile program.

    DRAM tensors:
      noise_h [64, nb, Bc] fp16   - lane-major noise
      init_t  [64, Bc]     fp32r  - initial window, lane-major (oldest first)
      w       [128, 64]    fp32r  - block-transition weights (lhsT)
      y_d     [64, nb, Bc] fp32r  - lane-major outputs
    """
    from concourse import bacc
    import concourse.mybir as mybir
    from concourse.tile import TileContext

    assert T % _TB == 0 and Bc % chunk == 0
    nb = T // _TB
    nchunks = Bc // chunk

    nc = bacc.Bacc("TRN2", target_bir_lowering=False, debug=False)
    f32 = mybir.dt.float32
    f32r = mybir.dt.float32r
    f16 = mybir.dt.float16
    noise_h = nc.dram_tensor("noise_h", [_TB, nb, Bc], f16, kind="ExternalInput")
    init_t = nc.dram_tensor("init_t", [_TB, Bc], f32r, kind="ExternalInput")
    w = nc.dram_tensor("w", [2 * _TB, _TB], f32r, kind="ExternalInput")
    y_d = nc.dram_tensor("y_d", [_TB, nb, Bc], f32r, kind="ExternalOutput")

    with TileContext(nc) as tc:
        with tc.tile_pool(name="wpool", bufs=1) as wpool, \
             tc.tile_pool(name="rhs", bufs=bufs) as rhspool, \
             tc.tile_pool(name="stage", bufs=bufs) as stagepool, \
             tc.tile_pool(name="ps", bufs=2, space="PSUM") as pspool:
            wt = wpool.tile([2 * _TB, _TB], f32r, tag="wt", name="wt")
            nc.sync.dma_start(out=wt[:, :], in_=w[:, :])

            # One rhs tile per block [128, Bc]: rows 0:64 = state (previous
            # block's outputs, also the store staging), rows 64:128 = this
            # block's noise. The noise arrives fp16 (halves the load-stream
            # bytes) into a staging tile on the SP HW-DGE ring; DVE/ACT
            # cast-copy it to fp32r three blocks ahead of use, in [64,512]
            # pieces interleaved one-per-chain-copy so a conversion never
            # delays the chain-critical PSUM->SBUF copies by more than
            # ~0.5us in the engine FIFOs.
            def alloc_rhs(b):
                t = rhspool.tile([2 * _TB, Bc], f32r, tag="rhs", name="rhs")
                st = None
                if b < nb:
                    st = stagepool.tile([2 * _TB, Bc], f16, tag="stage",
                                        name="stage")
                    nc.sync.dma_start(out=st[_TB:, :], in_=noise_h[:, b, :])
                return t, st

            tiles, stages = {}, {}
            for i in range(3):
                tiles[i], stages[i] = alloc_rhs(i)
            nc.sync.dma_start(out=tiles[0][0:_TB, :], in_=init_t[:, :])

            def emit_conv_piece(b, c):
                cs = slice(c * chunk, (c + 1) * chunk)
                src = stages[b][_TB:, cs]
                dst = tiles[b][_TB:, cs]
                if c % 2 == 0:
                    nc.vector.tensor_copy(out=dst, in_=src)
                else:
                    nc.scalar.copy(out=dst, in_=src)

            for bb in range(3):
                for c in range(nchunks):
                    emit_conv_piece(bb, c)

            for b in range(nb):
                cur = tiles[b]
                if b + 3 <= nb:
                    tiles[b + 3], stages[b + 3] = alloc_rhs(b + 3)
                nxt = tiles[b + 1]
                pss = []
                for c in range(nchunks):
                    cs = slice(c * chunk, (c + 1) * chunk)
                    ps = pspool.tile([_TB, chunk], f32, tag=f"ps{c}",
                                     name=f"ps{c}")
                    # Single K=128 float32r matmul (PE single-pass).
                    nc.tensor.matmul(
                        out=ps[:, :], lhsT=wt[:, :], rhs=cur[:, cs],
                        start=True, stop=True,
                    )
                    pss.append(ps)
                for c in range(nchunks):
                    cs = slice(c * chunk, (c + 1) * chunk)
                    # Chain-critical PSUM->SBUF copy, split across DVE/ACT.
                    if c % 2 == 0:
                        nc.vector.tensor_copy(out=nxt[0:_TB, cs],
                                              in_=pss[c][:, :])
                    else:
                        nc.scalar.copy(out=nxt[0:_TB, cs], in_=pss[c][:, :])
                # Conversion pieces for block b+3 AFTER all chain copies:
                # the ~0.5us pieces then run inside the next block's matmul
                # window instead of delaying the block-closing copies in
                # the engine FIFOs.
                if b + 3 < nb:
                    for c in range(nchunks):
                        emit_conv_piece(b + 3, c)
                # Store block b's outputs (staged as block b+1's state rows).
                # Issue from the otherwise-idle SP sequencer to keep the
                # copy engines' instruction streams short.
                nc.sync.dma_start(out=y_d[:, b, :], in_=nxt[0:_TB, :])
                del tiles[b]
                stages.pop(b, None)

    nc.compile()
    return nc


def _shard_inputs(initial_values, coefficients, log_noise_std, noise):
    B, T = noise.shape
    Bc = B // _NCORES
    nb = T // _TB
    sigma = float(np.exp(np.float64(np.asarray(log_noise_std))))
    W = _round_f32r(_build_weights(coefficients, sigma))
    noise_tf = np.asarray(noise, np.float32).T.astype(np.float16)  # [T, B]
    init_tf = _round_f32r(np.asarray(initial_values, np.float32).T)
    # lane-major: [T, B] -> [nb, 64, B] -> [64, nb, B]
    noise_lane = np.ascontiguousarray(
        noise_tf.reshape(nb, _TB, B).transpose(1, 0, 2))
    in_maps = []
    for i in range(_NCORES):
        cols = slice(i * Bc, (i + 1) * Bc)
        in_maps.append({
            "noise_h": np.ascontiguousarray(noise_lane[:, :, cols]),
            "init_t": np.ascontiguousarray(init_tf[:, cols]),
            "w": W,
        })
    return in_maps


def _run(initial_values, coefficients, log_noise_std, noise, trace=False):
    from concourse.bass_utils import run_bass_kernel_spmd

    B, T = noise.shape
    Bc = B // _NCORES
    chunk = 512 if Bc % 512 == 0 else Bc
    nc = _build_nc(T, Bc, chunk)
    in_maps = _shard_inputs(initial_values, coefficients, log_noise_std, noise)
    res = run_bass_kernel_spmd(
        nc, in_maps, core_ids=list(range(_NCORES)), trace=trace
    )
    # y_d [64, nb, Bc] lane-major -> [T, Bc]
    y_cores = [
        r["y_d"].transpose(1, 0, 2).reshape(T, Bc) for r in res.results
    ]
    y_t = np.concatenate(y_cores, axis=1)  # [T, B]
    out = np.ascontiguousarray(y_t.T)
    return out, res


def kernel(initial_values, coefficients, log_noise_std, noise, steps):
    steps = int(np.asarray(steps))
    noise = np.asarray(noise)
    assert noise.shape[1] == steps, (noise.shape, steps)
    out, _ = _run(initial_values, coefficients, log_noise_std, noise)
    return out


# revision 39
# speedup vs baseline: 1.0429x; 1.0071x over previous
"""AR(64) sampling kernel for Trainium2 (8 NeuronCores, batch-sharded).

Problem: x_t = sum_k c_k x_{t-64+k} + sigma * eps_t over 4096 steps for
16384 independent batch rows (64 lags).

Approach: the recurrence is linear, so a block of 64 consecutive outputs
is an exact linear function of (the previous 64 outputs, the block's 64
noise values):

    y_block = AS @ state + (sigma*AE) @ eps        (per batch column)

AS/AE are built on the host from the coefficients by running the
recurrence with unit initial conditions / unit impulses (exact linear
algebra, a few thousand host flops). On device, each core processes
2048 batch rows (time-major layout) as a chain of 64 blocks x 4
batch-chunks of 512: per (block, chunk) one K=128 float32r PE matmul
(lhsT = [AS; sigma*AE], rhs = [state; noise] assembled in one SBUF
tile), then a PSUM->SBUF copy of the result (which is simultaneously
the next block's state rows and the DMA-out staging), with streaming
DMA of noise in / outputs out.

float32r runs the PE single-pass (4x the fp32 matmul rate; fp32 needs 2
half-speed passes) at a 12-bit-mantissa input rounding (rel ~1.2e-4);
measured end-to-end error is ~7e-4 of the output absmax vs the fp32
reference, while fp32's ~6e-7 costs ~15% more wall time - this kernel
is DMA-bound either way (~225 GB/s aggregate measured per core).

DMA layout: noise / outputs live in DRAM lane-major ([64 lane, n_blocks,
Bc]) so every transfer is 8KB contiguous per SBUF partition. Loads ride
the SP HW-DGE ring, stores the ACT ring.
"""

import sys

import numpy as np

_TRN_REPO = "/opt/trn_rl_repo"
if _TRN_REPO not in sys.path:
    sys.path.insert(0, _TRN_REPO)

_TB = 64  # time-block size == number of AR lags
_NCORES = 8


def _build_weights(coefficients: np.ndarray, sigma: float) -> np.ndarray:
    """Exact [2n, n] block-transition weights from AR coefficients.

    Returns lhsT with lhsT.T @ [state; eps] = y_block, where state is the
    previous 64 outputs (oldest first) and eps the block's raw noise.
    """
    c = np.asarray(coefficients, dtype=np.float64)
    n = c.shape[0]
    assert n == _TB

    # AS[i, r] = d y_i / d state_r : simulate with window = unit vectors.
    win = np.eye(n, dtype=np.float64)  # rows: unit-state cases
    AS = np.empty((_TB, n), dtype=np.float64)
    for i in range(_TB):
        x = win @ c
        AS[i] = x
        win = np.concatenate([win[:, 1:], x[:, None]], axis=1)

    # AE[i, j] = d y_i / d eps_j : simulate unit impulses, zero init.
    win = np.zeros((_TB, n), dtype=np.float64)
    AE = np.empty((_TB, _TB), dtype=np.float64)
    for i in range(_TB):
        x = win @ c
        x[i] += 1.0
        AE[i] = x
        win = np.concatenate([win[:, 1:], x[:, None]], axis=1)

    W = np.concatenate([AS.T, float(sigma) * AE.T], axis=0)  # [2n, TB]
    return np.ascontiguousarray(W.astype(np.float32))


def blocked_numpy(initial_values, coefficients, log_noise_std, noise):
    """Host-side blocked simulation (same math the device runs); for testing."""
    sigma = float(np.exp(np.float64(np.asarray(log_noise_std))))
    W = _build_weights(coefficients, sigma)
    B, T = noise.shape
    y = np.empty((B, T), np.float32)
    state = np.asarray(initial_values, np.float32)
    for b in range(T // _TB):
        rhs = np.concatenate([state.T, noise[:, b * _TB:(b + 1) * _TB].T], axis=0)
        out = (W.T @ rhs.astype(np.float32)).astype(np.float32)  # [TB, B]
        y[:, b * _TB:(b + 1) * _TB] = out.T
        state = out.T
    return y


def _round_f32r(x: np.ndarray) -> np.ndarray:
    """Round fp32 values to the fp32r matmul datapath precision (mantissa
    rounded at bit 12), round-to-nearest-even - bit-exact vs walrus's
    fp32_to_fp32r."""
    u = np.ascontiguousarray(x, np.float32).view(np.uint32)
    low = u & np.uint32(0xFFF)
    base = u >> np.uint32(12)
    add = (low > 0x800) | ((low == 0x800) & ((base & 1) == 1))
    r = ((base + add.astype(np.uint32)) << np.uint32(12)).astype(np.uint32)
    return r.view(np.float32)


def _build_nc(T: int, Bc: int, chunk: int, bufs: int = 8):
    """Build the per-core Bass/Tile program.

    DRAM tensors:
      noise_h [64, nb, Bc] fp16   - lane-major noise
      init_t  [64, Bc]     fp32r  - initial window, lane-major (oldest first)
      w       [128, 64]    fp32r  - block-transition weights (lhsT)
      y_d     [64, nb, Bc] fp32r  - lane-major outputs
    """
    from concourse import bacc
    import concourse.mybir as mybir
    from concourse.tile import TileContext

    assert T % _TB == 0 and Bc % chunk == 0
    nb = T // _TB
    nchunks = Bc // chunk

    nc = bacc.Bacc("TRN2", target_bir_lowering=False, debug=False)
    f32 = mybir.dt.float32
    f32r = mybir.dt.float32r
    f16 = mybir.dt.float16
    noise_h = nc.dram_tensor("noise_h", [_TB, nb, Bc], f16, kind="ExternalInput")
    init_t = nc.dram_tensor("init_t", [_TB, Bc], f32r, kind="ExternalInput")
    w = nc.dram_tensor("w", [2 * _TB, _TB], f32r, kind="ExternalInput")
    y_d = nc.dram_tensor("y_d", [_TB, nb, Bc], f32r, kind="ExternalOutput")

    with TileContext(nc) as tc:
        with tc.tile_pool(name="wpool", bufs=1) as wpool, \
             tc.tile_pool(name="rhs", bufs=bufs) as rhspool, \
             tc.tile_pool(name="stage", bufs=bufs) as stagepool, \
             tc.tile_pool(name="ps", bufs=2, space="PSUM") as pspool:
            wt = wpool.tile([2 * _TB, _TB], f32r, tag="wt", name="wt")
            nc.sync.dma_start(out=wt[:, :], in_=w[:, :])

            # One rhs tile per block [128, Bc]: rows 0:64 = state (previous
            # block's outputs, also the store staging), rows 64:128 = this
            # block's noise. The noise arrives fp16 (halves the load-stream
            # bytes) into a staging tile on the SP HW-DGE ring; DVE/ACT
            # cast-copy it to fp32r three blocks ahead of use, in [64,512]
            # pieces interleaved one-per-chain-copy so a conversion never
            # delays the chain-critical PSUM->SBUF copies by more than
            # ~0.5us in the engine FIFOs.
            def alloc_rhs(b):
                t = rhspool.tile([2 * _TB, Bc], f32r, tag="rhs", name="rhs")
                st = None
                if b < nb:
                    st = stagepool.tile([2 * _TB, Bc], f16, tag="stage",
                                        name="stage")
                    nc.sync.dma_start(out=st[_TB:, :], in_=noise_h[:, b, :])
                return t, st

            tiles, stages = {}, {}
            for i in range(3):
                tiles[i], stages[i] = alloc_rhs(i)
            nc.sync.dma_start(out=tiles[0][0:_TB, :], in_=init_t[:, :])

            def emit_conv_piece(b, c):
                cs = slice(c * chunk, (c + 1) * chunk)
                src = stages[b][_TB:, cs]
                dst = tiles[b][_TB:, cs]
                if c % 2 == 0:
                    nc.vector.tensor_copy(out=dst, in_=src)
                else:
                    nc.scalar.copy(out=dst, in_=src)

            for bb in range(3):
                for c in range(nchunks):
                    emit_conv_piece(bb, c)

            for b in range(nb):
                cur = tiles[b]
                if b + 3 <= nb:
                    tiles[b + 3], stages[b + 3] = alloc_rhs(b + 3)
                nxt = tiles[b + 1]
                pss = []
                for c in range(nchunks):
                    cs = slice(c * chunk, (c + 1) * chunk)
                    ps = pspool.tile([_TB, chunk], f32, tag=f"ps{c}",
                                     name=f"ps{c}")
                    # Single K=128 float32r matmul (PE single-pass).
                    nc.tensor.matmul(
                        out=ps[:, :], lhsT=wt[:, :], rhs=cur[:, cs],
                        start=True, stop=True,
                    )
                    pss.append(ps)
                for c in range(nchunks):
                    cs = slice(c * chunk, (c + 1) * chunk)
                    # Chain-critical PSUM->SBUF copy, split across DVE/ACT.
                    if c % 2 == 0:
                        nc.vector.tensor_copy(out=nxt[0:_TB, cs],
                                              in_=pss[c][:, :])
                    else:
                        nc.scalar.copy(out=nxt[0:_TB, cs], in_=pss[c][:, :])
                # Conversion pieces for block b+3 AFTER all chain copies:
                # the ~0.5us pieces then run inside the next block's matmul
                # window instead of delaying the block-closing copies in
                # the engine FIFOs.
                if b + 3 < nb:
                    for c in range(nchunks):
                        emit_conv_piece(b + 3, c)
                # Store block b's outputs (staged as block b+1's state rows).
                # Issue from the otherwise-idle SP sequencer to keep the
                # copy engines' instruction streams short.
                nc.sync.dma_start(out=y_d[:, b, :], in_=nxt[0:_TB, :])
                del tiles[b]
                stages.pop(b, None)

    nc.compile()
    return nc


def _shard_inputs(initial_values, coefficients, log_noise_std, noise):
    B, T = noise.shape
    Bc = B // _NCORES
    nb = T // _TB
    sigma = float(np.exp(np.float64(np.asarray(log_noise_std))))
    W = _round_f32r(_build_weights(coefficients, sigma))
    noise_tf = np.asarray(noise, np.float32).T.astype(np.float16)  # [T, B]
    init_tf = _round_f32r(np.asarray(initial_values, np.float32).T)
    # lane-major: [T, B] -> [nb, 64, B] -> [64, nb, B]
    noise_lane = np.ascontiguousarray(
        noise_tf.reshape(nb, _TB, B).transpose(1, 0, 2))
    in_maps = []
    for i in range(_NCORES):
        cols = slice(i * Bc, (i + 1) * Bc)
        in_maps.append({
            "noise_h": np.ascontiguousarray(noise_lane[:, :, cols]),
            "init_t": np.ascontiguousarray(init_tf[:, cols]),
            "w": W,
        })
    return in_maps


def _run(initial_values, coefficients, log_noise_std, noise, trace=False):
    from concourse.bass_utils import run_bass_kernel_spmd

    B, T = noise.shape
    Bc = B // _NCORES
    chunk = 512 if Bc % 512 == 0 else Bc
    nc = _build_nc(T, Bc, chunk)
    in_maps = _shard_inputs(initial_values, coefficients, log_noise_std, noise)
    res = run_bass_kernel_spmd(
        nc, in_maps, core_ids=list(range(_NCORES)), trace=trace
    )
    # y_d [64, nb, Bc] lane-major -> [T, Bc]
    y_cores = [
        r["y_d"].transpose(1, 0, 2).reshape(T, Bc) for r in res.results
    ]
    y_t = np.concatenate(y_cores, axis=1)  # [T, B]
    out = np.ascontiguousarray(y_t.T)
    return out, res


def kernel(initial_values, coefficients, log_noise_std, noise, steps):
    steps = int(np.asarray(steps))
    noise = np.asarray(noise)
    assert noise.shape[1] == steps, (noise.shape, steps)
    out, _ = _run(initial_values, coefficients, log_noise_std, noise)
    return out
